# revision 1
# baseline (speedup 1.0000x reference)
"""BlockGrouper (MoE routing dispatch) Trainium2 kernel — raw bass.

Semantics (from the reference): each token n in sample b belongs to group
g = argmax(block_onehot[b, n]); its slot within the group is its rank
among same-group tokens in token order.  With the balanced one-hot
routing, the output [B, G, cap, D] is a pure row-permutation of
x [B, N, D].

Sharding: data-parallel over B across the 8 NeuronCores (one sample per
core); each core moves 16 MiB in + 16 MiB out (+16 MiB scatter-add RMW).

Per-core program (N=8192, G=16, D=512, cap=512, P=128, C=64; token n
lives at partition p = n // 64, column c = n % 64):
  1. Index pipeline: dest[n] = g*cap + rank(n)
     = sum_g onehot * (prefix_c + carry_p + g*cap - 1), where prefix_c is
     a per-partition inclusive prefix sum along c (16 strided
     tensor_tensor_scan ops) and carry_p comes from one
     strict-upper-triangular-ones matmul on the per-partition totals
     (plus a const-row matmul adding g*cap - 1).
  2. dest is folded into the SWDGE index layout (idx for scatter slot j
     lives at partition j%16, column j//16, replicated across the 8 Q7
     cores) with 8 replicated-selection matmuls + one strided DVE
     cast-copy to int16 — no DRAM bounce.
  3. Data path: 8 x-chunk loads (p-major, 16 KiB contiguous per
     partition, each chunk split across the SP and ACT HWDGE rings) and 8
     back-to-back 1024-index dma_scatter_add ops (one 2 KiB descriptor
     per row) into the zero-initialized output, spread over 4 SWDGE
     queues.  Raw bass (no Tile) so the scatters are not WAW-chained —
     they write disjoint rows.  A dummy 16-index scatter at t=0 preloads
     the Q7 scatter-library (saves ~13 us of lazy LOAD_LIB).
"""


import numpy as np

B, N, G, D = 8, 8192, 16, 512
CAP = N // G
P = 128
C = N // P
NCORES = 8
NCHUNK = 8
GCH = N // NCHUNK  # 1024

_cached = None


def _build():
    import concourse.bass as bass
    import concourse.bacc as bacc
    import concourse.mybir as mybir

    f32 = mybir.dt.float32
    i16 = mybir.dt.int16

    nc = bacc.Bacc("TRN2", target_bir_lowering=False, debug=False,
                   num_devices=NCORES, num_swdge_queues=4)
    x_d = nc.dram_tensor("x", [N, D], f32, kind="ExternalInput")
    oh_d = nc.dram_tensor("oh", [N, G], f32, kind="ExternalInput")
    cst_big_d = nc.dram_tensor("cst_big", [P, 9 * P], f32,
                               kind="ExternalInput")
    cst_row_d = nc.dram_tensor("cst_row", [1, P + G], f32,
                               kind="ExternalInput")
    out_d = nc.dram_tensor("out", [N, D], f32, kind="ExternalOutput")
    # tiny scratch target for the t=0 dummy scatter that preloads the Q7
    # scatter library (LOAD_LIB costs ~13us if taken lazily)
    dummy_d = nc.dram_tensor("lib_warm", [16, 64], f32, kind="ExternalOutput")

    with (
        nc.sbuf_tensor("cst_big_t", [P, 9 * P], f32) as cst_big_t,
        nc.sbuf_tensor("cst_row_t", [1, P + G], f32) as cst_row_t,
        nc.sbuf_tensor("oh_t", [P, C * G], f32) as oh_t,
        nc.sbuf_tensor("scan_t", [P, C * G], f32) as scan_t,
        nc.sbuf_tensor("s_t", [P, C * G], f32) as s_t,
        nc.sbuf_tensor("prod_t", [P, C * G], f32) as prod_t,
        nc.sbuf_tensor("dest_f", [P, C], f32) as dest_f,
        nc.sbuf_tensor("destw_t", [P, N // 16], i16) as destw_t,
        nc.sbuf_tensor("dummy_idx", [P, 1], i16) as dummy_idx,
        nc.sbuf_tensor("dummy_pay", [P, 1], f32) as dummy_pay,
        nc.sbuf_tensor("xt", [P, NCHUNK * (GCH // P) * D], f32) as xt,
        nc.psum_tensor("a_ps", [P, G], f32) as a_ps,
        nc.psum_tensor("ps_w", [P, C * 8], f32) as ps_w,
        nc.semaphore("s_const") as s_const,
        nc.semaphore("s_oh") as s_oh,
        nc.semaphore("s_xl_sp") as s_xl_sp,
        nc.semaphore("s_xl_act") as s_xl_act,
        nc.semaphore("s_scat") as s_scat,
        nc.semaphore("s_dve") as s_dve,
        nc.semaphore("s_pe") as s_pe,
    ):
        su_t = cst_big_t[:, 0:P]
        # repsel_t[t]: [128, 128] with [t*16+q, m*16+q] = 1 — the fold
        # matmul with it replicates dest across the 8 Q7 partition groups
        repsel = [cst_big_t[:, (1 + t) * P:(2 + t) * P] for t in range(8)]
        ones_t = cst_row_t[:, 0:P]
        cst_t = cst_row_t[:, P:P + G]

        # ---------------- plain DMAs ----------------
        # oh first on the SP ring (it gates the whole index pipeline);
        # constants on the ACT ring.
        nc.sync.dma_start(
            out=oh_t[:],
            in_=oh_d[:].rearrange("(p c) g -> p (c g)", p=P)).then_inc(
            s_oh, 16)
        nc.scalar.dma_start(out=cst_big_t[:], in_=cst_big_d[:]).then_inc(
            s_const, 16)
        nc.scalar.dma_start(out=cst_row_t[:], in_=cst_row_d[:]).then_inc(
            s_const, 16)
        # p-major: scatter slot j carries x row (j%128)*64 + j//128,
        # matching the destw fold; 16 KiB contiguous per partition per chunk
        # Loads alternate between the two HWDGE rings (SP / ACT) so early
        # chunks finish fast on both and nothing queues behind them.
        x3 = x_d[:].rearrange("(p c) d -> p c d", p=P)
        cc = GCH // P  # 8
        xto = xt[:].rearrange("p (c d) -> p c d", d=D)
        for k in range(NCHUNK):
            h = cc // 2
            nc.sync.dma_start(
                out=xto[:, k * cc:k * cc + h, :],
                in_=x3[:, k * cc:k * cc + h, :]).then_inc(s_xl_sp, 16)
            nc.scalar.dma_start(
                out=xto[:, k * cc + h:(k + 1) * cc, :],
                in_=x3[:, k * cc + h:(k + 1) * cc, :]).then_inc(s_xl_act, 16)

        # ---------------- DVE ----------------
        nc.vector.wait_ge(s_oh, 16)
        for g in range(G):
            ins = nc.vector.tensor_tensor_scan(
                out=scan_t[:, g::G], data0=oh_t[:, g::G],
                data1=oh_t[:, g::G], initial=0.0,
                op0=mybir.AluOpType.add, op1=mybir.AluOpType.bypass)
            if g == G - 1:
                ins.then_inc(s_dve, 1)
        nc.vector.wait_ge(s_pe, 1)
        a_bcast = a_ps[:].unsqueeze(1).to_broadcast([P, C, G])
        nc.vector.tensor_tensor(
            out=s_t[:].rearrange("p (c g) -> p c g", g=G),
            in0=scan_t[:].rearrange("p (c g) -> p c g", g=G),
            in1=a_bcast, op=mybir.AluOpType.add)
        nc.vector.tensor_tensor(out=prod_t[:], in0=oh_t[:], in1=s_t[:],
                                op=mybir.AluOpType.mult)
        nc.vector.tensor_reduce(
            out=dest_f[:],
            in_=prod_t[:].rearrange("p (c g) -> p c g", g=G),
            axis=mybir.AxisListType.X,
            op=mybir.AluOpType.add).then_inc(s_dve, 1)
        nc.vector.wait_ge(s_pe, 2)
        nc.vector.tensor_copy(
            out=destw_t[:].rearrange("q (c t) -> q c t", t=8),
            in_=ps_w[:].rearrange("q (t c) -> q c t", c=C)).then_inc(
            s_dve, 1)

        # ---------------- PE ----------------
        nc.tensor.wait_ge(s_const, 32)
        nc.tensor.wait_ge(s_dve, 1)
        rowtot = scan_t[:, (C - 1) * G: C * G]
        nc.tensor.matmul(out=a_ps[:], lhsT=su_t, rhs=rowtot,
                         start=True, stop=False)
        nc.tensor.matmul(out=a_ps[:], lhsT=ones_t, rhs=cst_t,
                         start=False, stop=True).then_inc(s_pe, 1)
        nc.tensor.wait_ge(s_dve, 2)
        for t in range(8):
            ins = nc.tensor.matmul(out=ps_w[:, t * C:(t + 1) * C],
                                   lhsT=repsel[t],
                                   rhs=dest_f[:], start=True, stop=True)
        ins.then_inc(s_pe, 1)

        # ---------------- Pool: scatter-adds ----------------
        # dummy 16-idx scatter at t=0: pulls LOAD_LIB + warms the path
        nc.gpsimd.memset(dummy_idx[:], 0)
        nc.gpsimd.dma_scatter_add(
            dummy_d[:][:, 0:1],
            dummy_pay[:].rearrange("p (c one) -> p c one", one=1),
            dummy_idx[:], 16, 16, 1, elem_step=64).then_inc(s_scat, 16)
        nidx_reg = nc.gpsimd.to_reg(GCH)
        nc.gpsimd.wait_ge(s_dve, 3)  # destw written + replicated
        xt3 = xt[:].rearrange("p (c d) -> p c d", d=D)
        for k in range(NCHUNK):
            nc.gpsimd.wait_ge(s_xl_sp, 16 * (k + 1))
            nc.gpsimd.wait_ge(s_xl_act, 16 * (k + 1))
            nc.gpsimd.dma_scatter_add(
                out_d[:],
                xt3[:, k * cc:(k + 1) * cc, :],
                destw_t[:, k * (GCH // 16):(k + 1) * (GCH // 16)],
                GCH, nidx_reg, D, queue_num=k % 4).then_inc(s_scat, 16)
        nc.gpsimd.wait_ge(s_scat, 16 * (NCHUNK + 1))

    nc.compile()
    return nc


def _get_nc():
    global _cached
    if _cached is None:
        _cached = _build()
    return _cached


def _constants():
    su = np.triu(np.ones((P, P), np.float32), k=1)
    rs = []
    for t in range(8):
        m = np.zeros((P, P), np.float32)
        for mm in range(8):
            for q in range(16):
                m[t * 16 + q, mm * 16 + q] = 1.0
        rs.append(m)
    cst_big = np.concatenate([su] + rs, axis=1)
    ones_r = np.ones((1, P), np.float32)
    cst = (np.arange(G, dtype=np.float32) * CAP - 1.0).reshape(1, G)
    cst_row = np.concatenate([ones_r, cst], axis=1)
    return cst_big, cst_row


def kernel(x, block_onehot, capacity):
    from concourse.bass_utils import run_bass_kernel_spmd

    x = np.ascontiguousarray(np.asarray(x, dtype=np.float32))
    oh = np.asarray(block_onehot, dtype=np.float32)
    if oh.ndim == 2:
        oh = np.broadcast_to(oh[None], (B,) + oh.shape)
    oh = np.ascontiguousarray(oh)
    assert x.shape == (B, N, D), x.shape
    assert oh.shape == (B, N, G), oh.shape
    assert int(capacity) == CAP, capacity
    nc = _get_nc()
    cst_big, cst_row = _constants()
    in_maps = [
        {"x": x[b], "oh": oh[b], "cst_big": cst_big, "cst_row": cst_row}
        for b in range(B)
    ]
    res = run_bass_kernel_spmd(nc, in_maps, core_ids=list(range(NCORES)))
    return np.stack([res.results[b]["out"].reshape(G, CAP, D)
                     for b in range(B)])



# revision 10
# speedup vs baseline: 1.5530x; 1.5530x over previous
"""BlockGrouper (MoE routing dispatch) Trainium2 kernel — raw bass.

Semantics (from the reference): each token n in sample b belongs to group
g = argmax(block_onehot[b, n]); its slot within the group is its rank
among same-group tokens in token order.  With the balanced one-hot
routing, the output [B, G, cap, D] is a pure row-permutation of
x [B, N, D].

Sharding: data-parallel over B across the 8 NeuronCores (one sample per
core); each core moves 16 MiB in + 16 MiB out (+16 MiB scatter-add RMW).

Per-core program (N=8192, G=16, D=512, cap=512, P=128, C=64; token n
lives at partition p = n // 64, column c = n % 64):
  1. Index pipeline: dest[n] = g*cap + rank(n)
     = sum_g onehot * (prefix_c + carry_p + g*cap - 1), where prefix_c is
     a per-partition inclusive prefix sum along c (16 strided
     tensor_tensor_scan ops) and carry_p comes from one
     strict-upper-triangular-ones matmul on the per-partition totals
     (plus a const-row matmul adding g*cap - 1).
  2. dest is folded into the SWDGE index layout (idx for scatter slot j
     lives at partition j%16, column j//16, replicated across the 8 Q7
     cores) with 8 replicated-selection matmuls + one strided DVE
     cast-copy to int16 — no DRAM bounce.
  3. Data path: 8 x-chunk loads (p-major, 16 KiB contiguous per
     partition, each chunk split across the SP and ACT HWDGE rings) into
     a 4-deep f32 ring buffer, an ACT-engine f32->bf16 cast per chunk,
     and 8 back-to-back 1024-index bf16 dma_scatter_add ops (one 1 KiB
     descriptor per row) into the zero-initialized bf16 output, spread
     over 4 SWDGE queues.  Raw bass (no Tile) so the scatters are not
     WAW-chained — they write disjoint rows.  A dummy 16-index scatter
     at t=0 preloads the Q7 scatter-library (saves ~13 us of lazy
     LOAD_LIB).

     bf16 payload: the scatter-add's CCE path reads the old HBM value
     (RMW) in addition to the SBUF payload, so M2S bytes = 2x payload;
     halving the payload dtype cuts the dominant scatter cost in half.
     Each output row is written exactly once onto zeros, so the add is
     exact; bf16 rounding (<= 2^-9 rel) is far inside the 2e-2 gate.
     The host upcasts the bf16 result back to float32.
"""


import numpy as np

B, N, G, D = 8, 8192, 16, 512
CAP = N // G
P = 128
C = N // P
NCORES = 8
NCHUNK = 8
GCH = N // NCHUNK  # 1024

_cached = None


def _build():
    import concourse.bass as bass
    import concourse.bacc as bacc
    import concourse.mybir as mybir

    f32 = mybir.dt.float32
    bf16 = mybir.dt.bfloat16
    i16 = mybir.dt.int16

    nc = bacc.Bacc("TRN2", target_bir_lowering=False, debug=False,
                   num_devices=NCORES, num_swdge_queues=4)
    x_d = nc.dram_tensor("x", [N, D], f32, kind="ExternalInput")
    oh_d = nc.dram_tensor("oh", [N, G], f32, kind="ExternalInput")
    cst_big_d = nc.dram_tensor("cst_big", [P, 9 * P], f32,
                               kind="ExternalInput")
    cst_row_d = nc.dram_tensor("cst_row", [1, P + G], f32,
                               kind="ExternalInput")
    out_d = nc.dram_tensor("out", [N, D], bf16, kind="ExternalOutput")
    # tiny scratch target for the t=0 dummy scatter that preloads the Q7
    # scatter library (LOAD_LIB costs ~13us if taken lazily)
    dummy_d = nc.dram_tensor("lib_warm", [16, 64], f32, kind="ExternalOutput")

    from contextlib import ExitStack
    with ExitStack() as ctx:
        cst_big_t = ctx.enter_context(nc.sbuf_tensor("cst_big_t", [P, 9 * P], f32))
        cst_row_t = ctx.enter_context(nc.sbuf_tensor("cst_row_t", [1, P + G], f32))
        oh_t = ctx.enter_context(nc.sbuf_tensor("oh_t", [P, C * G], f32))
        scan_t = ctx.enter_context(nc.sbuf_tensor("scan_t", [P, C * G], f32))
        s_t = ctx.enter_context(nc.sbuf_tensor("s_t", [P, C * G], f32))
        prod_t = ctx.enter_context(nc.sbuf_tensor("prod_t", [P, C * G], f32))
        dest_f = ctx.enter_context(nc.sbuf_tensor("dest_f", [P, C], f32))
        destw_t = ctx.enter_context(nc.sbuf_tensor("destw_t", [P, N // 16], i16))
        dummy_idx = ctx.enter_context(nc.sbuf_tensor("dummy_idx", [P, 1], i16))
        dummy_pay = ctx.enter_context(nc.sbuf_tensor("dummy_pay", [P, 1], f32))
        xbuf = ctx.enter_context(nc.sbuf_tensor("xbuf", [P, 4 * (GCH // P) * D], f32))
        xtb = ctx.enter_context(nc.sbuf_tensor("xtb", [P, NCHUNK * (GCH // P) * D], bf16))
        a_ps = ctx.enter_context(nc.psum_tensor("a_ps", [P, G], f32))
        ps_w = ctx.enter_context(nc.psum_tensor("ps_w", [P, C * 8], f32))
        s_const = ctx.enter_context(nc.semaphore("s_const"))
        s_oh = ctx.enter_context(nc.semaphore("s_oh"))
        s_xl_sp = ctx.enter_context(nc.semaphore("s_xl_sp"))
        s_xl_act = ctx.enter_context(nc.semaphore("s_xl_act"))
        s_scat = ctx.enter_context(nc.semaphore("s_scat"))
        s_cast = ctx.enter_context(nc.semaphore("s_cast"))
        s_dve = ctx.enter_context(nc.semaphore("s_dve"))
        s_pe = ctx.enter_context(nc.semaphore("s_pe"))
        su_t = cst_big_t[:, 0:P]
        # repsel_t[t]: [128, 128] with [t*16+q, m*16+q] = 1 — the fold
        # matmul with it replicates dest across the 8 Q7 partition groups
        repsel = [cst_big_t[:, (1 + t) * P:(2 + t) * P] for t in range(8)]
        ones_t = cst_row_t[:, 0:P]
        cst_t = cst_row_t[:, P:P + G]

        # ---------------- plain DMAs ----------------
        # oh first on the SP ring (it gates the whole index pipeline);
        # constants on the ACT ring.
        nc.sync.dma_start(
            out=oh_t[:],
            in_=oh_d[:].rearrange("(p c) g -> p (c g)", p=P)).then_inc(
            s_oh, 16)
        nc.scalar.dma_start(out=cst_big_t[:], in_=cst_big_d[:]).then_inc(
            s_const, 16)
        nc.scalar.dma_start(out=cst_row_t[:], in_=cst_row_d[:]).then_inc(
            s_const, 16)
        # p-major: scatter slot j carries x row (j%128)*64 + j//128,
        # matching the destw fold; 16 KiB contiguous per partition per chunk
        # Loads alternate between the two HWDGE rings (SP / ACT) so early
        # chunks finish fast on both and nothing queues behind them.
        # f32 chunks land in a 4-deep ring; the ACT engine casts each to
        # bf16 (xtb) which is what the scatters read.
        x3 = x_d[:].rearrange("(p c) d -> p c d", p=P)
        cc = GCH // P  # 8
        NBUF = 4
        xbo = xbuf[:].rearrange("p (c d) -> p c d", d=D)
        for k in range(NCHUNK):
            h = cc // 2
            s = (k % NBUF) * cc
            if k >= NBUF:
                # ring reuse: wait for the cast of the chunk that lived here
                nc.sync.wait_ge(s_cast, k - NBUF + 1)
            nc.sync.dma_start(
                out=xbo[:, s:s + h, :],
                in_=x3[:, k * cc:k * cc + h, :]).then_inc(s_xl_sp, 16)
            nc.scalar.dma_start(
                out=xbo[:, s + h:s + cc, :],
                in_=x3[:, k * cc + h:(k + 1) * cc, :]).then_inc(s_xl_act, 16)
            if k >= NBUF - 1:
                # cast chunk k-NBUF+1 (interleaved so ACT-queue program
                # order alone already guards the ACT-half ring reuse)
                kc = k - NBUF + 1
                nc.scalar.wait_ge(s_xl_sp, 16 * (kc + 1))
                nc.scalar.wait_ge(s_xl_act, 16 * (kc + 1))
                nc.scalar.copy(
                    out=xtb[:, kc * cc * D:(kc + 1) * cc * D],
                    in_=xbuf[:, (kc % NBUF) * cc * D:
                             ((kc % NBUF) + 1) * cc * D]).then_inc(s_cast, 1)
        for kc in range(NCHUNK - NBUF + 1, NCHUNK):
            nc.scalar.wait_ge(s_xl_sp, 16 * (kc + 1))
            nc.scalar.wait_ge(s_xl_act, 16 * (kc + 1))
            nc.scalar.copy(
                out=xtb[:, kc * cc * D:(kc + 1) * cc * D],
                in_=xbuf[:, (kc % NBUF) * cc * D:
                         ((kc % NBUF) + 1) * cc * D]).then_inc(s_cast, 1)

        # ---------------- DVE ----------------
        nc.vector.wait_ge(s_oh, 16)
        for g in range(G):
            ins = nc.vector.tensor_tensor_scan(
                out=scan_t[:, g::G], data0=oh_t[:, g::G],
                data1=oh_t[:, g::G], initial=0.0,
                op0=mybir.AluOpType.add, op1=mybir.AluOpType.bypass)
            if g == G - 1:
                ins.then_inc(s_dve, 1)
        nc.vector.wait_ge(s_pe, 1)
        a_bcast = a_ps[:].unsqueeze(1).to_broadcast([P, C, G])
        nc.vector.tensor_tensor(
            out=s_t[:].rearrange("p (c g) -> p c g", g=G),
            in0=scan_t[:].rearrange("p (c g) -> p c g", g=G),
            in1=a_bcast, op=mybir.AluOpType.add)
        nc.vector.tensor_tensor(out=prod_t[:], in0=oh_t[:], in1=s_t[:],
                                op=mybir.AluOpType.mult)
        nc.vector.tensor_reduce(
            out=dest_f[:],
            in_=prod_t[:].rearrange("p (c g) -> p c g", g=G),
            axis=mybir.AxisListType.X,
            op=mybir.AluOpType.add).then_inc(s_dve, 1)
        nc.vector.wait_ge(s_pe, 2)
        nc.vector.tensor_copy(
            out=destw_t[:].rearrange("q (c t) -> q c t", t=8),
            in_=ps_w[:].rearrange("q (t c) -> q c t", c=C)).then_inc(
            s_dve, 1)

        # ---------------- PE ----------------
        nc.tensor.wait_ge(s_const, 32)
        nc.tensor.wait_ge(s_dve, 1)
        rowtot = scan_t[:, (C - 1) * G: C * G]
        nc.tensor.matmul(out=a_ps[:], lhsT=su_t, rhs=rowtot,
                         start=True, stop=False)
        nc.tensor.matmul(out=a_ps[:], lhsT=ones_t, rhs=cst_t,
                         start=False, stop=True).then_inc(s_pe, 1)
        nc.tensor.wait_ge(s_dve, 2)
        for t in range(8):
            ins = nc.tensor.matmul(out=ps_w[:, t * C:(t + 1) * C],
                                   lhsT=repsel[t],
                                   rhs=dest_f[:], start=True, stop=True)
        ins.then_inc(s_pe, 1)

        # ---------------- Pool: scatter-adds ----------------
        # dummy 16-idx scatter at t=0: pulls LOAD_LIB + warms the path
        nc.gpsimd.memset(dummy_idx[:], 0)
        nc.gpsimd.dma_scatter_add(
            dummy_d[:][:, 0:1],
            dummy_pay[:].rearrange("p (c one) -> p c one", one=1),
            dummy_idx[:], 16, 16, 1, elem_step=64).then_inc(s_scat, 16)
        nidx_reg = nc.gpsimd.to_reg(GCH)
        nc.gpsimd.wait_ge(s_dve, 3)  # destw written + replicated
        xtb3 = xtb[:].rearrange("p (c d) -> p c d", d=D)
        for k in range(NCHUNK):
            nc.gpsimd.wait_ge(s_cast, k + 1)
            nc.gpsimd.dma_scatter_add(
                out_d[:],
                xtb3[:, k * cc:(k + 1) * cc, :],
                destw_t[:, k * (GCH // 16):(k + 1) * (GCH // 16)],
                GCH, nidx_reg, D, queue_num=k % 4).then_inc(s_scat, 16)
        nc.gpsimd.wait_ge(s_scat, 16 * (NCHUNK + 1))

    nc.compile()
    return nc


def _get_nc():
    global _cached
    if _cached is None:
        _cached = _build()
    return _cached


def _constants():
    su = np.triu(np.ones((P, P), np.float32), k=1)
    rs = []
    for t in range(8):
        m = np.zeros((P, P), np.float32)
        for mm in range(8):
            for q in range(16):
                m[t * 16 + q, mm * 16 + q] = 1.0
        rs.append(m)
    cst_big = np.concatenate([su] + rs, axis=1)
    ones_r = np.ones((1, P), np.float32)
    cst = (np.arange(G, dtype=np.float32) * CAP - 1.0).reshape(1, G)
    cst_row = np.concatenate([ones_r, cst], axis=1)
    return cst_big, cst_row


def kernel(x, block_onehot, capacity):
    from concourse.bass_utils import run_bass_kernel_spmd

    x = np.ascontiguousarray(np.asarray(x, dtype=np.float32))
    oh = np.asarray(block_onehot, dtype=np.float32)
    if oh.ndim == 2:
        oh = np.broadcast_to(oh[None], (B,) + oh.shape)
    oh = np.ascontiguousarray(oh)
    assert x.shape == (B, N, D), x.shape
    assert oh.shape == (B, N, G), oh.shape
    assert int(capacity) == CAP, capacity
    nc = _get_nc()
    cst_big, cst_row = _constants()
    in_maps = [
        {"x": x[b], "oh": oh[b], "cst_big": cst_big, "cst_row": cst_row}
        for b in range(B)
    ]
    res = run_bass_kernel_spmd(nc, in_maps, core_ids=list(range(NCORES)))
    return np.stack([np.asarray(res.results[b]["out"])
                     .astype(np.float32).reshape(G, CAP, D)
                     for b in range(B)])

